# revision 37
# baseline (speedup 1.0000x reference)
"""Trainium2 Bass kernel: additive (Bahdanau-style) attention readout.

Reference computation (per batch b):
    energy  = tanh(enc @ W1.T + dec_b @ W2.T + W_b)      # (S, H)
    scores  = energy @ V + V_b, masked                   # (S,)
    attn    = softmax(scores)                            # (S,)
    context = attn @ enc                                 # (D,)

Sharding: data-parallel over batch across 8 NeuronCores (4 batches/core),
small weights replicated.

Numerics: pass1 runs on the PE in fp8-e4m3 with the DoubleRow perf mode
(256-deep contraction per matmul at 0.5 cycles/row).  The fp8
quantization error of enc and W1 is compensated with a host-side rank-1
correction folded into the additive score penalty:
    scores_err ~= e_dev @ (dW @ (V*cbar_b)) + dE @ (W1T @ (V*cbar_b))
where cbar_b[h] = E_s[tanh'(X_sh)] via Gauss-Hermite over the (b,h)
pre-activation distribution.  Measured rel-err ~8e-3 (gate 2e-2).

Device dataflow per batch:
  - et tiles [p=d, k, s] (fp8, x16) stream on the SP HWDGE queue.
  - pass1: psum[h_m, s_n] = sum_k W8-pair.T @ et-pair  (fp8 DoubleRow)
  - tanh on ScalarE (psum*(1/1024) + cbias -> fp16 energy, all m in one
    [P, 8, 512] tile per token tile).
  - scores via FLIPPED matmuls: stationary = energy [128h,128s] slice,
    moving = V-chunk [128h, 1] -> psum column [128s, 1]; per-column
    groups accumulate over m (m inner => sequential column groups in
    the shared 2KB zero region, which is the legal order).
  - softmax in [128, 16] partition-major layout, NO max subtraction
    (scores are bounded by ||V||; masked tokens carry -1e30 penalty and
    exp underflows to 0): DVE adds penalty, ScalarE exp w/ accum,
    partition-sum via a ones-column matmul, DVE reciprocal, and a
    1x128-ones matmul broadcasts 1/Z to all partitions.
  - pass2 (context) via flipped matmuls on natural-layout fp16 enc
    streamed on the ACT queue: stationary = enc chunk [128s, 128d],
    moving = unnormalized attn [128s, 1]; per-j-chunk psum partials are
    accumulated on DVE, scaled by 1/Z, and DMA'd out.

Cost model (per core): PE ~113 us busy; the serialized DMA stream
(~52 MB: 16.8 MB fp8 enc-T + 33.5 MB fp16 enc-N + 2 MB W) is the
roofline at ~146 us busy; measured span ~153 us (vs 552 us for the
fp16 full-pass baseline).
"""

import numpy as np
import ml_dtypes

import concourse.bass as bass
import concourse.tile as tile
from concourse import bacc, mybir
from concourse.bass_utils import run_bass_kernel_spmd

# Problem shapes (hardcoded per contract).
B, S, D, H = 32, 2048, 2048, 1024
NCORES = 8
BPC = B // NCORES  # batches per core

SE = 16.0   # fp8 scale for enc
SW = 64.0   # fp8 scale for W1

F32 = mybir.dt.float32
F16 = mybir.dt.float16
F8 = mybir.dt.float8e4
F8NP = ml_dtypes.float8_e4m3
AF = mybir.ActivationFunctionType
ALU = mybir.AluOpType
PM = mybir.MatmulPerfMode


def build_program(bpc=BPC, s=S, d=D, h=H, nt=512, skg=4, encn_bufs=5,
                  etch=4, et_bufs=3, smalls_q="sync", w1_q="gpsimd",
                  lag=1, mm_bufs=2):
    P = 128
    KD = d // P            # enc-feature 128-chunks (16)
    KD2 = KD // 2          # DoubleRow 256-chunks (8)
    MH = h // P            # h chunks (8)
    NT = s // nt           # token tiles (4)
    SK = s // P            # s 128-chunks (16)
    NJ = SK // skg         # natural-layout j-chunks per batch
    nhalf = 2
    sh = s // nhalf        # tokens per transposed-tile half
    NTH = sh // nt         # token tiles per half

    nc = bacc.Bacc(None, target_bir_lowering=False)
    enc8 = nc.declare_dram_parameter("enc8", [bpc, d, s], F8, isOutput=False)
    w1 = nc.declare_dram_parameter("w1", [d, h], F8, isOutput=False)
    # All small constants packed in one [P, 256] f32 blob (single full-rate
    # DMA instead of several tiny strided ones):
    #   cols [0 : bpc*SK)            pen, batch-major
    #   cols [64 : 64+MH*bpc)        cbias, m-major
    #   cols [96 : 96+MH)            V chunks (f32; moving-operand cost of
    #                                the flipped scores matmul is ~1 cycle
    #                                regardless of dtype)
    #   col  104                     ones column (partition reduce)
    #   cols [112 : 240)             ones row (1/Z broadcast stationary)
    NBLOB = 256
    blob = nc.declare_dram_parameter("blob", [P, NBLOB], F32, isOutput=False)
    encn = nc.declare_dram_parameter("encn", [bpc, s, d], F16, isOutput=False)
    # [b, p, c] layout: contiguous DMA from the [P, SK] ctx tile; the host
    # re-interleaves to [b, d] (d = c*128 + p) after gather.
    ctx_out = nc.declare_dram_parameter(
        "ctx", [bpc, P, d // P], F32, isOutput=True
    )

    with tile.TileContext(nc) as tc:
        with (
            tc.tile_pool(name="singles", bufs=1) as singles,
            tc.tile_pool(name="et_pool", bufs=et_bufs) as et_pool,
            tc.tile_pool(name="en_pool", bufs=lag + 1) as en_pool,
            tc.tile_pool(name="encn_pool", bufs=encn_bufs) as encn_pool,
            tc.tile_pool(name="sc_pool", bufs=2) as sc_pool,
            tc.tile_pool(name="attn_pool", bufs=2) as attn_pool,
            tc.tile_pool(name="stat_pool", bufs=2) as stat_pool,
            tc.tile_pool(name="ctxa_pool", bufs=2) as ctxa_pool,
            tc.tile_pool(name="ctxo_pool", bufs=2) as ctxo_pool,
            tc.tile_pool(name="psum_mm", bufs=mm_bufs, space="PSUM") as psum_mm,
            tc.tile_pool(name="psum_sc", bufs=2, space="PSUM") as psum_sc,
            tc.tile_pool(name="psum_ctx", bufs=2, space="PSUM") as psum_ctx,
            tc.tile_pool(name="psum_rbz", bufs=1, space="PSUM") as psum_rbz,
        ):
            sq = getattr(nc, smalls_q)
            wq = getattr(nc, w1_q)
            blob_sb = singles.tile([P, NBLOB], F32)
            sq.dma_start(blob_sb, blob[:, :])

            def pen_col(b):
                return blob_sb[:, b * SK:(b + 1) * SK]

            def cb_col(m, b):
                c0 = 64 + m * bpc + b
                return blob_sb[:, c0:c0 + 1]

            vt_sb = singles.tile([P, MH], F16)
            nc.vector.tensor_copy(vt_sb, blob_sb[:, 96:96 + MH])

            def vt_col(m):
                return vt_sb[:, m:m + 1]

            oc_sb = blob_sb[:, 104:105]
            or_sb = blob_sb[0:1, 112:240]
            w1_sb = singles.tile([P, KD, h], F8)
            w1_r = w1.rearrange("(ko p) hh -> p ko hh", p=P)
            for k in range(0, KD, 4):
                wq.dma_start(w1_sb[:, k:k + 4, :], w1_r[:, k:k + 4, :])

            et_tiles = {}    # (b, hf) -> tile
            en_tiles = {}    # (b, t) -> tile
            encn_tiles = {}  # (b, j) -> tile
            psc_of = {}      # b -> psum scores tile
            p1_done = set()

            def load_et(b, hf):
                et = et_pool.tile([P, KD, sh], F8, tag="et")
                for kc in range(0, KD, etch):
                    nc.sync.dma_start(
                        et[:, kc:kc + etch, :],
                        enc8[
                            b, kc * P:(kc + etch) * P, hf * sh:(hf + 1) * sh
                        ].rearrange("(ko p) t -> p ko t", p=P),
                    )
                et_tiles[(b, hf)] = et

            def load_encn(b, j):
                ent = encn_pool.tile([P, skg, d], F16, tag="ent")
                nc.scalar.dma_start(
                    ent,
                    encn[
                        b, j * skg * P:(j + 1) * skg * P, :
                    ].rearrange("(c p) dd -> p c dd", p=P),
                )
                encn_tiles[(b, j)] = ent

            def emit_p1(b, t):
                if (b, t) in p1_done:
                    return
                p1_done.add((b, t))
                hf, n = t // NTH, t % NTH
                if (b, hf) not in et_tiles:
                    load_et(b, hf)
                et = et_tiles[(b, hf)]
                en = en_pool.tile([P, MH, nt], F16, tag="en")
                for m in range(MH):
                    ps = psum_mm.tile([P, nt], F32, tag="mm")
                    for c in range(KD2):
                        nc.tensor.matmul(
                            ps,
                            w1_sb[:, 2 * c:2 * c + 2, m * P:(m + 1) * P],
                            et[:, 2 * c:2 * c + 2, n * nt:(n + 1) * nt],
                            start=(c == 0),
                            stop=(c == KD2 - 1),
                            perf_mode=PM.DoubleRow,
                        )
                    nc.scalar.activation(
                        en[:, m, :], ps, AF.Tanh,
                        bias=cb_col(m, b), scale=1.0 / (SE * SW),
                    )
                en_tiles[(b, t)] = en
                if t == NTH - 1 and hf == 0:
                    # queue the second half's loads right after the first
                    # half's last tile is emitted
                    pass

            def emit_sc(b, t):
                if b not in psc_of:
                    psc_of[b] = psum_sc.tile(
                        [P, SK], F32, tag="psc", name="psc"
                    )
                psc = psc_of[b]
                en = en_tiles.pop((b, t))
                for j in range(nt // P):
                    sk = t * (nt // P) + j
                    for m in range(MH):
                        nc.tensor.matmul(
                            psc[:, sk:sk + 1],
                            en[:, m, j * P:(j + 1) * P],
                            vt_col(m),
                            start=(m == 0),
                            stop=(m == MH - 1),
                        )

            attn_of = {}
            rb_of = {}

            def emit_softmax(b):
                # softmax (no max subtraction: scores are small by
                # construction; masked tokens have -1e30 penalty -> exp 0)
                sc_sb = sc_pool.tile([P, SK], F32, tag="sc")
                nc.vector.tensor_tensor(
                    sc_sb, psc_of.pop(b), pen_col(b), ALU.add
                )
                attn = attn_pool.tile([P, SK], F16, tag="attn")
                esum = stat_pool.tile([P, 1], F32, tag="esum")
                nc.scalar.activation(
                    attn, sc_sb, AF.Exp, scale=1.0, accum_out=esum
                )
                rbz = psum_rbz.tile([P, 2], F32, tag="rbz")
                # Z = sum over partitions (ones-column stationary)
                nc.tensor.matmul(rbz[0:1, 1:2], oc_sb, esum)
                rinv = stat_pool.tile([1, 1], F32, tag="rinv")
                nc.vector.reciprocal(rinv, rbz[0:1, 1:2])
                # broadcast 1/Z to all 128 partitions; park it in SBUF so
                # the psum slot recycles before the (possibly much later)
                # pass2 consumes it
                nc.tensor.matmul(rbz[:, 0:1], or_sb, rinv)
                rb = stat_pool.tile([P, 1], F32, tag="rb", bufs=4)
                nc.vector.tensor_copy(rb, rbz[:, 0:1])
                attn_of[b] = attn
                rb_of[b] = rb

            def emit_pass2(b):
                attn = attn_of.pop(b)
                ctxacc = ctxa_pool.tile([P, SK], F32, tag="ctxa")
                for j in range(NJ):
                    ent = encn_tiles.pop((b, j))
                    ctp = psum_ctx.tile([P, SK], F32, tag="ctp")
                    for cd in range(SK):
                        for q in range(skg):
                            sk = j * skg + q
                            nc.tensor.matmul(
                                ctp[:, cd:cd + 1],
                                ent[:, q, cd * P:(cd + 1) * P],
                                attn[:, sk:sk + 1],
                                start=(q == 0),
                                stop=(q == skg - 1),
                            )
                    if j == 0:
                        nc.vector.tensor_copy(ctxacc, ctp)
                    else:
                        nc.vector.tensor_tensor(ctxacc, ctxacc, ctp, ALU.add)
                ctxo = ctxo_pool.tile([P, SK], F32, tag="ctxo")
                nc.vector.tensor_scalar_mul(ctxo, ctxacc, rb_of.pop(b))
                nc.sync.dma_start(ctx_out[b], ctxo)

            # Global tile schedule: pass1 runs LAG tiles ahead of the
            # flipped scores matmuls so ScalarE's tanh latency (and the
            # per-batch softmax chain) hides under queued pass1 work.
            LAG = lag
            tiles = [(b, t) for b in range(bpc) for t in range(NT)]

            def emit_done(i):
                bb, tt = tiles[i]
                emit_sc(bb, tt)
                if tt == NT - 1:
                    emit_softmax(bb)
                    emit_pass2(bb)

            for i, (b, t) in enumerate(tiles):
                emit_p1(b, t)
                if t >= NT - 2:
                    # queue this batch's natural-layout loads after both
                    # transposed halves are on the SP queue
                    for _j in range((NJ // 2) * (t - (NT - 2)),
                                    (NJ // 2) * (t - (NT - 2) + 1)):
                        load_encn(b, _j)
                if i >= LAG:
                    emit_done(i - LAG)
            for i in range(len(tiles) - LAG, len(tiles)):
                emit_done(i)
    nc.finalize()
    return nc


_PROGRAM_CACHE = {}


def _get_program(key="full", **kwargs):
    if key not in _PROGRAM_CACHE:
        _PROGRAM_CACHE[key] = build_program(**kwargs)
    return _PROGRAM_CACHE[key]


def prep_inputs(enc_output, enc_mask, dec_hidden, W_w, W_b, V_w, V_b):
    """Host-side shard + prep: returns per-core in_maps."""
    P, SK = 128, S // 128
    enc = np.asarray(enc_output, dtype=np.float32)                 # (B,S,D)
    mask = np.asarray(enc_mask, dtype=np.float32)[..., 0]          # (B,S)
    dec = np.asarray(dec_hidden, dtype=np.float32)[0]              # (B,H)
    W = np.asarray(W_w, dtype=np.float32)                          # (H,3H)
    Wb = np.asarray(W_b, dtype=np.float32)                         # (H,)
    V = np.asarray(V_w, dtype=np.float32)[0]                       # (H,)
    Vb = float(np.asarray(V_b, dtype=np.float32)[0])

    w1t = np.ascontiguousarray(W[:, :D].T)                         # (D,H)
    cb = dec @ W[:, D:].T + Wb                                     # (B,H)

    W8 = (w1t * SW).astype(F8NP)                                   # (D,H)
    dW = W8.astype(np.float32) / SW - w1t                          # (D,H)

    # cbar[b,h] = E_s[tanh'(X_sh)] with X_sh ~ N(cb[b,h], sig^2),
    # via Gauss-Hermite quadrature.
    sig = float(np.sqrt(
        (enc.astype(np.float64) ** 2).mean()
        * (w1t.astype(np.float64) ** 2).sum(0).mean()
    ))
    xs, ws = np.polynomial.hermite.hermgauss(21)
    wsn = ws / np.sqrt(np.pi)
    z = np.sqrt(2.0) * sig * xs[None, None, :] + cb[:, :, None]
    cbar = (wsn * (1.0 / np.cosh(z) ** 2)).sum(-1).astype(np.float32)

    pen_all = np.where(mask > 0, 0.0, -1e30).astype(np.float32) + Vb
    vc = V[None, :] * cbar                                         # (B,H)
    gW = dW @ vc.T                                                 # (D,B)
    gE = w1t @ vc.T                                                # (D,B)

    enc8T = np.empty((B, D, S), dtype=F8NP)
    encn16 = enc.astype(np.float16)                                # (B,S,D)
    for b in range(B):
        e8b = (enc[b] * SE).astype(F8NP)                           # (S,D)
        enc8T[b] = e8b.T
        e_dev = e8b.astype(np.float32) * (1.0 / SE)                # (S,D)
        corr = e_dev @ (gW[:, b] + gE[:, b]) - enc[b] @ gE[:, b]
        pen_all[b] -= corr
    pen_dev = np.ascontiguousarray(
        pen_all.reshape(B, SK, P).transpose(0, 2, 1)               # (B,P,SK)
    )

    MH = H // P
    in_maps = []
    for c in range(NCORES):
        sl = slice(c * BPC, (c + 1) * BPC)
        blob = np.zeros((P, 256), dtype=np.float32)
        # pen: cols [0 : BPC*SK), batch-major
        blob[:, :BPC * SK] = pen_dev[sl].transpose(1, 0, 2).reshape(P, -1)
        # cbias: cols [64 : 64+MH*BPC), m-major; cb_sb[p, m, b] = cb[b, m*P+p]
        cb_core = cb[sl].T.reshape(MH, P, BPC).transpose(1, 0, 2)  # (P,MH,BPC)
        blob[:, 64:64 + MH * BPC] = cb_core.reshape(P, -1)
        # V chunks: cols [96 : 96+MH); vt[p, m] = V[m*P+p]
        blob[:, 96:96 + MH] = V.reshape(MH, P).T
        blob[:, 104] = 1.0       # ones column
        blob[:, 112:240] = 1.0   # ones row (read from partition 0)
        in_maps.append({
            "enc8": enc8T[sl],
            "w1": W8,
            "blob": blob,
            "encn": encn16[sl],
        })
    return in_maps


def kernel(**inputs) -> np.ndarray:
    in_maps = prep_inputs(**inputs)
    nc = _get_program("full")
    res = run_bass_kernel_spmd(nc, in_maps, list(range(NCORES)))
    out = np.concatenate(
        [res.results[c]["ctx"] for c in range(NCORES)], axis=0
    )                                           # (B, P, D//P)
    out = out.transpose(0, 2, 1).reshape(B, D)  # d = c*128 + p
    return np.ascontiguousarray(out.astype(np.float32))


if __name__ == "__main__":
    rng = np.random.default_rng(0)
    inputs = {
        "enc_output": rng.standard_normal((B, S, D), dtype=np.float32),
        "enc_mask": np.ones((B, S, 1), dtype=np.float32),
        "dec_hidden": rng.standard_normal((1, B, H), dtype=np.float32),
        "W_w": (rng.standard_normal((H, 3 * H), dtype=np.float32)
                / np.sqrt(3 * H)),
        "W_b": np.zeros((H,), dtype=np.float32),
        "V_w": rng.standard_normal((1, H), dtype=np.float32) / np.sqrt(H),
        "V_b": np.zeros((1,), dtype=np.float32),
    }
    out = kernel(**inputs)
    print(out.shape, out.dtype, float(np.abs(out).mean()))


# revision 43
# speedup vs baseline: 1.0148x; 1.0148x over previous
"""Trainium2 Bass kernel: additive (Bahdanau-style) attention readout.

Reference computation (per batch b):
    energy  = tanh(enc @ W1.T + dec_b @ W2.T + W_b)      # (S, H)
    scores  = energy @ V + V_b, masked                   # (S,)
    attn    = softmax(scores)                            # (S,)
    context = attn @ enc                                 # (D,)

Sharding: data-parallel over batch across 8 NeuronCores (4 batches/core),
small weights replicated.

Numerics: pass1 runs on the PE in fp8-e4m3 with the DoubleRow perf mode
(256-deep contraction per matmul at 0.5 cycles/row).  The fp8
quantization error of enc and W1 is compensated with a host-side rank-1
correction folded into the additive score penalty:
    scores_err ~= e_dev @ (dW @ (V*cbar_b)) + dE @ (W1T @ (V*cbar_b))
where cbar_b[h] = E_s[tanh'(X_sh)] via Gauss-Hermite over the (b,h)
pre-activation distribution.  Measured rel-err ~8e-3 (gate 2e-2).

Device dataflow per batch:
  - et tiles [p=d, k, s] (fp8, x16) stream on the SP HWDGE queue.
  - pass1: psum[h_m, s_n] = sum_k W8-pair.T @ et-pair  (fp8 DoubleRow)
  - tanh on ScalarE (psum*(1/1024) + cbias -> fp16 energy, all m in one
    [P, 8, 512] tile per token tile).
  - scores via FLIPPED matmuls: stationary = energy [128h,128s] slice,
    moving = V-chunk [128h, 1] -> psum column [128s, 1]; per-column
    groups accumulate over m (m inner => sequential column groups in
    the shared 2KB zero region, which is the legal order).
  - softmax in [128, 16] partition-major layout, NO max subtraction
    (scores are bounded by ||V||; masked tokens carry -1e30 penalty and
    exp underflows to 0): DVE adds penalty, ScalarE exp w/ accum,
    partition-sum via a ones-column matmul, DVE reciprocal, and a
    1x128-ones matmul broadcasts 1/Z to all partitions.
  - pass2 (context) via flipped matmuls on natural-layout fp16 enc
    streamed on the ACT queue: stationary = enc chunk [128s, 128d],
    moving = unnormalized attn [128s, 1]; per-j-chunk psum partials are
    accumulated on DVE, scaled by 1/Z, and DMA'd out.

Cost model (per core): PE ~113 us busy; the serialized DMA stream
(~52 MB: 16.8 MB fp8 enc-T + 33.5 MB fp16 enc-N + 2 MB W) is the
roofline at ~146 us busy; measured span ~153 us (vs 552 us for the
fp16 full-pass baseline).
"""

import numpy as np
import ml_dtypes

import concourse.bass as bass
import concourse.tile as tile
from concourse import bacc, mybir
from concourse.bass_utils import run_bass_kernel_spmd

# Problem shapes (hardcoded per contract).
B, S, D, H = 32, 2048, 2048, 1024
NCORES = 8
BPC = B // NCORES  # batches per core

SE = 16.0   # fp8 e4m3 scale for enc (pass1)
SW = 64.0   # fp8 e4m3 scale for W1
SN = 3.0    # fp8 e3m4 scale for natural-layout enc (pass2)

F32 = mybir.dt.float32
F16 = mybir.dt.float16
F8 = mybir.dt.float8e4
F83 = mybir.dt.float8e3
F8NP = ml_dtypes.float8_e4m3
F83NP = ml_dtypes.float8_e3m4
AF = mybir.ActivationFunctionType
ALU = mybir.AluOpType
PM = mybir.MatmulPerfMode


def build_program(bpc=BPC, s=S, d=D, h=H, nt=512, skg=4, encn_bufs=5,
                  etch=4, et_bufs=3, smalls_q="sync", w1_q="gpsimd",
                  lag=1, mm_bufs=2):
    P = 128
    KD = d // P            # enc-feature 128-chunks (16)
    KD2 = KD // 2          # DoubleRow 256-chunks (8)
    MH = h // P            # h chunks (8)
    NT = s // nt           # token tiles (4)
    SK = s // P            # s 128-chunks (16)
    NJ = SK // skg         # natural-layout j-chunks per batch
    nhalf = 2
    sh = s // nhalf        # tokens per transposed-tile half
    NTH = sh // nt         # token tiles per half

    nc = bacc.Bacc(None, target_bir_lowering=False)
    enc8 = nc.declare_dram_parameter("enc8", [bpc, d, s], F8, isOutput=False)
    w1 = nc.declare_dram_parameter("w1", [d, h], F8, isOutput=False)
    # All small constants packed in one [P, 256] f32 blob (single full-rate
    # DMA instead of several tiny strided ones):
    #   cols [0 : bpc*SK)            pen, batch-major
    #   cols [64 : 64+MH*bpc)        cbias, m-major
    #   cols [96 : 96+MH)            V chunks (f32; moving-operand cost of
    #                                the flipped scores matmul is ~1 cycle
    #                                regardless of dtype)
    #   col  104                     ones column (partition reduce)
    #   cols [112 : 240)             ones row (1/Z broadcast stationary)
    NBLOB = 256
    blob = nc.declare_dram_parameter("blob", [P, NBLOB], F32, isOutput=False)
    # natural-layout enc for pass2 in fp8-e3m4 (x3, clipped): 4 mantissa
    # bits give ~1.2e-2 context error -- inside the budget -- at HALF the
    # fp16 DMA footprint, which is what sets the kernel's roofline
    encn = nc.declare_dram_parameter("encn", [bpc, s, d], F83, isOutput=False)
    # [b, p, c] layout: contiguous DMA from the [P, SK] ctx tile; the host
    # re-interleaves to [b, d] (d = c*128 + p) after gather.
    ctx_out = nc.declare_dram_parameter(
        "ctx", [bpc, P, d // P], F32, isOutput=True
    )

    with tile.TileContext(nc) as tc:
        with (
            tc.tile_pool(name="singles", bufs=1) as singles,
            tc.tile_pool(name="et_pool", bufs=et_bufs) as et_pool,
            tc.tile_pool(name="en_pool", bufs=lag + 1) as en_pool,
            tc.tile_pool(name="encn_pool", bufs=encn_bufs) as encn_pool,
            tc.tile_pool(name="sc_pool", bufs=2) as sc_pool,
            tc.tile_pool(name="attn_pool", bufs=2) as attn_pool,
            tc.tile_pool(name="stat_pool", bufs=2) as stat_pool,
            tc.tile_pool(name="ctxa_pool", bufs=2) as ctxa_pool,
            tc.tile_pool(name="ctxo_pool", bufs=2) as ctxo_pool,
            tc.tile_pool(name="psum_mm", bufs=mm_bufs, space="PSUM") as psum_mm,
            tc.tile_pool(name="psum_sc", bufs=2, space="PSUM") as psum_sc,
            tc.tile_pool(name="psum_ctx", bufs=2, space="PSUM") as psum_ctx,
            tc.tile_pool(name="psum_rbz", bufs=1, space="PSUM") as psum_rbz,
        ):
            sq = getattr(nc, smalls_q)
            wq = getattr(nc, w1_q)
            blob_sb = singles.tile([P, NBLOB], F32)
            sq.dma_start(blob_sb, blob[:, :])

            def pen_col(b):
                return blob_sb[:, b * SK:(b + 1) * SK]

            def cb_col(m, b):
                c0 = 64 + m * bpc + b
                return blob_sb[:, c0:c0 + 1]

            vt_sb = singles.tile([P, MH], F16)
            nc.vector.tensor_copy(vt_sb, blob_sb[:, 96:96 + MH])

            def vt_col(m):
                return vt_sb[:, m:m + 1]

            oc_sb = blob_sb[:, 104:105]
            or_sb = blob_sb[0:1, 112:240]
            w1_sb = singles.tile([P, KD, h], F8)
            w1_r = w1.rearrange("(ko p) hh -> p ko hh", p=P)
            for k in range(0, KD, 4):
                wq.dma_start(w1_sb[:, k:k + 4, :], w1_r[:, k:k + 4, :])

            et_tiles = {}    # (b, hf) -> tile
            en_tiles = {}    # (b, t) -> tile
            encn_tiles = {}  # (b, j) -> tile
            psc_of = {}      # b -> psum scores tile
            p1_done = set()

            def load_et(b, hf):
                et = et_pool.tile([P, KD, sh], F8, tag="et")
                for kc in range(0, KD, etch):
                    nc.sync.dma_start(
                        et[:, kc:kc + etch, :],
                        enc8[
                            b, kc * P:(kc + etch) * P, hf * sh:(hf + 1) * sh
                        ].rearrange("(ko p) t -> p ko t", p=P),
                    )
                et_tiles[(b, hf)] = et

            def load_encn(b, j):
                ent = encn_pool.tile([P, skg, d], F83, tag="ent")
                nc.scalar.dma_start(
                    ent,
                    encn[
                        b, j * skg * P:(j + 1) * skg * P, :
                    ].rearrange("(c p) dd -> p c dd", p=P),
                )
                encn_tiles[(b, j)] = ent

            def emit_p1(b, t):
                if (b, t) in p1_done:
                    return
                p1_done.add((b, t))
                hf, n = t // NTH, t % NTH
                if (b, hf) not in et_tiles:
                    load_et(b, hf)
                et = et_tiles[(b, hf)]
                en = en_pool.tile([P, MH, nt], F16, tag="en")
                for m in range(MH):
                    ps = psum_mm.tile([P, nt], F32, tag="mm")
                    for c in range(KD2):
                        nc.tensor.matmul(
                            ps,
                            w1_sb[:, 2 * c:2 * c + 2, m * P:(m + 1) * P],
                            et[:, 2 * c:2 * c + 2, n * nt:(n + 1) * nt],
                            start=(c == 0),
                            stop=(c == KD2 - 1),
                            perf_mode=PM.DoubleRow,
                        )
                    nc.scalar.activation(
                        en[:, m, :], ps, AF.Tanh,
                        bias=cb_col(m, b), scale=1.0 / (SE * SW),
                    )
                en_tiles[(b, t)] = en
                if t == NTH - 1 and hf == 0:
                    # queue the second half's loads right after the first
                    # half's last tile is emitted
                    pass

            def emit_sc(b, t):
                if b not in psc_of:
                    psc_of[b] = psum_sc.tile(
                        [P, SK], F32, tag="psc", name="psc"
                    )
                psc = psc_of[b]
                en = en_tiles.pop((b, t))
                for j in range(nt // P):
                    sk = t * (nt // P) + j
                    for m in range(MH):
                        nc.tensor.matmul(
                            psc[:, sk:sk + 1],
                            en[:, m, j * P:(j + 1) * P],
                            vt_col(m),
                            start=(m == 0),
                            stop=(m == MH - 1),
                        )

            attn_of = {}
            rb_of = {}

            def emit_softmax(b):
                # softmax (no max subtraction: scores are small by
                # construction; masked tokens have -1e30 penalty -> exp 0)
                sc_sb = sc_pool.tile([P, SK], F32, tag="sc")
                nc.vector.tensor_tensor(
                    sc_sb, psc_of.pop(b), pen_col(b), ALU.add
                )
                attn = attn_pool.tile([P, SK], F16, tag="attn")
                esum = stat_pool.tile([P, 1], F32, tag="esum")
                nc.scalar.activation(
                    attn, sc_sb, AF.Exp, scale=1.0, accum_out=esum
                )
                rbz = psum_rbz.tile([P, 2], F32, tag="rbz")
                # Z = sum over partitions (ones-column stationary)
                nc.tensor.matmul(rbz[0:1, 1:2], oc_sb, esum)
                rinv = stat_pool.tile([1, 1], F32, tag="rinv")
                nc.vector.reciprocal(rinv, rbz[0:1, 1:2])
                # broadcast 1/Z to all 128 partitions; park it in SBUF so
                # the psum slot recycles before the (possibly much later)
                # pass2 consumes it
                nc.tensor.matmul(rbz[:, 0:1], or_sb, rinv)
                rb = stat_pool.tile([P, 1], F32, tag="rb", bufs=4)
                nc.vector.tensor_copy(rb, rbz[:, 0:1])
                attn_of[b] = attn
                rb_of[b] = rb

            def emit_pass2(b):
                attn = attn_of.pop(b)
                ctxacc = ctxa_pool.tile([P, SK], F32, tag="ctxa")
                for j in range(NJ):
                    ent = encn_tiles.pop((b, j))
                    ctp = psum_ctx.tile([P, SK], F32, tag="ctp")
                    for cd in range(SK):
                        for q in range(skg):
                            sk = j * skg + q
                            nc.tensor.matmul(
                                ctp[:, cd:cd + 1],
                                ent[:, q, cd * P:(cd + 1) * P],
                                attn[:, sk:sk + 1],
                                start=(q == 0),
                                stop=(q == skg - 1),
                            )
                    if j == 0:
                        nc.vector.tensor_copy(ctxacc, ctp)
                    else:
                        nc.vector.tensor_tensor(ctxacc, ctxacc, ctp, ALU.add)
                ctxo = ctxo_pool.tile([P, SK], F32, tag="ctxo")
                nc.vector.tensor_scalar_mul(ctxo, ctxacc, rb_of.pop(b))
                nc.sync.dma_start(ctx_out[b], ctxo)

            # Global tile schedule: pass1 runs LAG tiles ahead of the
            # flipped scores matmuls so ScalarE's tanh latency (and the
            # per-batch softmax chain) hides under queued pass1 work.
            LAG = lag
            tiles = [(b, t) for b in range(bpc) for t in range(NT)]

            def emit_done(i):
                bb, tt = tiles[i]
                emit_sc(bb, tt)
                if tt == NT - 1:
                    emit_softmax(bb)
                    emit_pass2(bb)

            for i, (b, t) in enumerate(tiles):
                emit_p1(b, t)
                if t >= NT - 2:
                    # queue this batch's natural-layout loads after both
                    # transposed halves are on the SP queue
                    for _j in range((NJ // 2) * (t - (NT - 2)),
                                    (NJ // 2) * (t - (NT - 2) + 1)):
                        load_encn(b, _j)
                if i >= LAG:
                    emit_done(i - LAG)
            for i in range(len(tiles) - LAG, len(tiles)):
                emit_done(i)
    nc.finalize()
    return nc


_PROGRAM_CACHE = {}


def _get_program(key="full", **kwargs):
    if key not in _PROGRAM_CACHE:
        _PROGRAM_CACHE[key] = build_program(**kwargs)
    return _PROGRAM_CACHE[key]


def prep_inputs(enc_output, enc_mask, dec_hidden, W_w, W_b, V_w, V_b):
    """Host-side shard + prep: returns per-core in_maps."""
    P, SK = 128, S // 128
    enc = np.asarray(enc_output, dtype=np.float32)                 # (B,S,D)
    mask = np.asarray(enc_mask, dtype=np.float32)[..., 0]          # (B,S)
    dec = np.asarray(dec_hidden, dtype=np.float32)[0]              # (B,H)
    W = np.asarray(W_w, dtype=np.float32)                          # (H,3H)
    Wb = np.asarray(W_b, dtype=np.float32)                         # (H,)
    V = np.asarray(V_w, dtype=np.float32)[0]                       # (H,)
    Vb = float(np.asarray(V_b, dtype=np.float32)[0])

    w1t = np.ascontiguousarray(W[:, :D].T)                         # (D,H)
    cb = dec @ W[:, D:].T + Wb                                     # (B,H)

    W8 = (w1t * SW).astype(F8NP)                                   # (D,H)
    dW = W8.astype(np.float32) / SW - w1t                          # (D,H)

    # cbar[b,h] = E_s[tanh'(X_sh)] with X_sh ~ N(cb[b,h], sig^2),
    # via Gauss-Hermite quadrature.
    sig = float(np.sqrt(
        (enc.astype(np.float64) ** 2).mean()
        * (w1t.astype(np.float64) ** 2).sum(0).mean()
    ))
    xs, ws = np.polynomial.hermite.hermgauss(21)
    wsn = ws / np.sqrt(np.pi)
    z = np.sqrt(2.0) * sig * xs[None, None, :] + cb[:, :, None]
    cbar = (wsn * (1.0 / np.cosh(z) ** 2)).sum(-1).astype(np.float32)

    pen_all = np.where(mask > 0, 0.0, -1e30).astype(np.float32) + Vb
    vc = V[None, :] * cbar                                         # (B,H)
    gW = dW @ vc.T                                                 # (D,B)
    gE = w1t @ vc.T                                                # (D,B)

    enc8T = np.empty((B, D, S), dtype=F8NP)
    mx = float(ml_dtypes.finfo(F83NP).max)
    encn8 = np.clip(enc * SN, -mx, mx).astype(F83NP)               # (B,S,D)
    for b in range(B):
        e8b = (enc[b] * SE).astype(F8NP)                           # (S,D)
        enc8T[b] = e8b.T
        e_dev = e8b.astype(np.float32) * (1.0 / SE)                # (S,D)
        corr = e_dev @ (gW[:, b] + gE[:, b]) - enc[b] @ gE[:, b]
        pen_all[b] -= corr
    pen_dev = np.ascontiguousarray(
        pen_all.reshape(B, SK, P).transpose(0, 2, 1)               # (B,P,SK)
    )

    MH = H // P
    in_maps = []
    for c in range(NCORES):
        sl = slice(c * BPC, (c + 1) * BPC)
        blob = np.zeros((P, 256), dtype=np.float32)
        # pen: cols [0 : BPC*SK), batch-major
        blob[:, :BPC * SK] = pen_dev[sl].transpose(1, 0, 2).reshape(P, -1)
        # cbias: cols [64 : 64+MH*BPC), m-major; cb_sb[p, m, b] = cb[b, m*P+p]
        cb_core = cb[sl].T.reshape(MH, P, BPC).transpose(1, 0, 2)  # (P,MH,BPC)
        blob[:, 64:64 + MH * BPC] = cb_core.reshape(P, -1)
        # V chunks: cols [96 : 96+MH); vt[p, m] = V[m*P+p]
        blob[:, 96:96 + MH] = V.reshape(MH, P).T
        blob[:, 104] = 1.0          # ones column (partition reduce)
        # "ones row": stationary of the 1/Z broadcast; carries the 1/SN
        # descale of the e3m4 natural-layout enc, so pass2's final
        # tensor_scalar_mul applies 1/(Z*SN) in one shot
        blob[:, 112:240] = 1.0 / SN
        in_maps.append({
            "enc8": enc8T[sl],
            "w1": W8,
            "blob": blob,
            "encn": encn8[sl],
        })
    return in_maps


def kernel(**inputs) -> np.ndarray:
    in_maps = prep_inputs(**inputs)
    nc = _get_program("full")
    res = run_bass_kernel_spmd(nc, in_maps, list(range(NCORES)))
    out = np.concatenate(
        [res.results[c]["ctx"] for c in range(NCORES)], axis=0
    )                                           # (B, P, D//P)
    out = out.transpose(0, 2, 1).reshape(B, D)  # d = c*128 + p
    return np.ascontiguousarray(out.astype(np.float32))


if __name__ == "__main__":
    rng = np.random.default_rng(0)
    inputs = {
        "enc_output": rng.standard_normal((B, S, D), dtype=np.float32),
        "enc_mask": np.ones((B, S, 1), dtype=np.float32),
        "dec_hidden": rng.standard_normal((1, B, H), dtype=np.float32),
        "W_w": (rng.standard_normal((H, 3 * H), dtype=np.float32)
                / np.sqrt(3 * H)),
        "W_b": np.zeros((H,), dtype=np.float32),
        "V_w": rng.standard_normal((1, H), dtype=np.float32) / np.sqrt(H),
        "V_b": np.zeros((1,), dtype=np.float32),
    }
    out = kernel(**inputs)
    print(out.shape, out.dtype, float(np.abs(out).mean()))


# revision 48
# speedup vs baseline: 1.1472x; 1.1305x over previous
"""Trainium2 Bass kernel: additive (Bahdanau-style) attention readout.

Reference computation (per batch b):
    energy  = tanh(enc @ W1.T + dec_b @ W2.T + W_b)      # (S, H)
    scores  = energy @ V + V_b, masked                   # (S,)
    attn    = softmax(scores)                            # (S,)
    context = attn @ enc                                 # (D,)

Sharding: data-parallel over batch across 8 NeuronCores (4 batches/core),
small weights replicated.

Numerics: pass1 runs on the PE in fp8-e4m3 with the DoubleRow perf mode
(256-deep contraction per matmul at 0.5 cycles/row).  The fp8
quantization error of enc and W1 is compensated with a host-side rank-1
correction folded into the additive score penalty:
    scores_err ~= e_dev @ (dW @ (V*cbar_b)) + dE @ (W1T @ (V*cbar_b))
where cbar_b[h] = E_s[tanh'(X_sh)] via Gauss-Hermite over the (b,h)
pre-activation distribution.  Measured rel-err ~8e-3 (gate 2e-2).

Device dataflow per batch:
  - et tiles [p=d, k, s] (fp8, x16) stream on the SP HWDGE queue.
  - pass1: psum[h_m, s_n] = sum_k W8-pair.T @ et-pair  (fp8 DoubleRow)
  - tanh on ScalarE (psum*(1/1024) + cbias -> fp16 energy, all m in one
    [P, 8, 512] tile per token tile).
  - scores via FLIPPED matmuls: stationary = energy [128h,128s] slice,
    moving = V-chunk [128h, 1] -> psum column [128s, 1]; per-column
    groups accumulate over m (m inner => sequential column groups in
    the shared 2KB zero region, which is the legal order).
  - softmax in [128, 16] partition-major layout, NO max subtraction
    (scores are bounded by ||V||; masked tokens carry -1e30 penalty and
    exp underflows to 0): DVE adds penalty, ScalarE exp w/ accum,
    partition-sum via a ones-column matmul, DVE reciprocal, and a
    1x128-ones matmul broadcasts 1/Z to all partitions.
  - pass2 (context) via flipped matmuls on natural-layout fp16 enc
    streamed on the ACT queue: stationary = enc chunk [128s, 128d],
    moving = unnormalized attn [128s, 1]; per-j-chunk psum partials are
    accumulated on DVE, scaled by 1/Z, and DMA'd out.

Cost model (per core): PE ~113 us busy; the serialized DMA stream
(~52 MB: 16.8 MB fp8 enc-T + 33.5 MB fp16 enc-N + 2 MB W) is the
roofline at ~146 us busy; measured span ~153 us (vs 552 us for the
fp16 full-pass baseline).
"""

import numpy as np
import ml_dtypes

import concourse.bass as bass
import concourse.tile as tile
from concourse import bacc, mybir
from concourse.bass_utils import run_bass_kernel_spmd

# Problem shapes (hardcoded per contract).
B, S, D, H = 32, 2048, 2048, 1024
NCORES = 8
BPC = B // NCORES  # batches per core

SE = 16.0   # fp8 e4m3 scale for enc (pass1)
SW = 64.0   # fp8 e4m3 scale for W1
SN = 3.0    # fp8 e3m4 scale for natural-layout enc (pass2)

F32 = mybir.dt.float32
F16 = mybir.dt.float16
F8 = mybir.dt.float8e4
F83 = mybir.dt.float8e3
F8NP = ml_dtypes.float8_e4m3
F83NP = ml_dtypes.float8_e3m4
AF = mybir.ActivationFunctionType
ALU = mybir.AluOpType
PM = mybir.MatmulPerfMode


def build_program(bpc=BPC, s=S, d=D, h=H, nt=512, skg=4, encn_bufs=5,
                  etch=4, et_bufs=3, smalls_q="gpsimd", w1_q="scalar",
                  lag=1, mm_bufs=3, ctx_bufs=2, head_ilv=False):
    P = 128
    KD = d // P            # enc-feature 128-chunks (16)
    KD2 = KD // 2          # DoubleRow 256-chunks (8)
    MH = h // P            # h chunks (8)
    NT = s // nt           # token tiles (4)
    SK = s // P            # s 128-chunks (16)
    NJ = SK // skg         # natural-layout j-chunks per batch
    nhalf = 2
    sh = s // nhalf        # tokens per transposed-tile half
    NTH = sh // nt         # token tiles per half

    nc = bacc.Bacc(None, target_bir_lowering=False)
    enc8 = nc.declare_dram_parameter("enc8", [bpc, d, s], F8, isOutput=False)
    w1 = nc.declare_dram_parameter("w1", [d, h], F8, isOutput=False)
    # All small constants packed in one [P, 256] f32 blob (single full-rate
    # DMA instead of several tiny strided ones):
    #   cols [0 : bpc*SK)            pen, batch-major
    #   cols [64 : 64+MH*bpc)        cbias, m-major
    #   cols [96 : 96+MH)            V chunks (f32; moving-operand cost of
    #                                the flipped scores matmul is ~1 cycle
    #                                regardless of dtype)
    #   col  104                     ones column (partition reduce)
    #   cols [112 : 240)             ones row (1/Z broadcast stationary)
    NBLOB = 256
    blob = nc.declare_dram_parameter("blob", [P, NBLOB], F32, isOutput=False)
    # natural-layout enc for pass2 in fp8-e3m4 (x3, clipped): 4 mantissa
    # bits give ~1.2e-2 context error -- inside the budget -- at HALF the
    # fp16 DMA footprint, which is what sets the kernel's roofline
    encn = nc.declare_dram_parameter("encn", [bpc, s, d], F83, isOutput=False)
    # [b, p, c] layout: contiguous DMA from the [P, SK] ctx tile; the host
    # re-interleaves to [b, d] (d = c*128 + p) after gather.
    ctx_out = nc.declare_dram_parameter(
        "ctx", [bpc, P, d // P], F32, isOutput=True
    )

    with tile.TileContext(nc) as tc:
        with (
            tc.tile_pool(name="singles", bufs=1) as singles,
            tc.tile_pool(name="et_pool", bufs=et_bufs) as et_pool,
            tc.tile_pool(name="en_pool", bufs=lag + 1) as en_pool,
            tc.tile_pool(name="encn_pool", bufs=encn_bufs) as encn_pool,
            tc.tile_pool(name="sc_pool", bufs=2) as sc_pool,
            tc.tile_pool(name="attn_pool", bufs=2) as attn_pool,
            tc.tile_pool(name="stat_pool", bufs=2) as stat_pool,
            tc.tile_pool(name="ctxa_pool", bufs=2) as ctxa_pool,
            tc.tile_pool(name="ctxo_pool", bufs=2) as ctxo_pool,
            tc.tile_pool(name="psum_mm", bufs=mm_bufs, space="PSUM") as psum_mm,
            tc.tile_pool(name="psum_sc", bufs=2, space="PSUM") as psum_sc,
            tc.tile_pool(
                name="psum_ctx", bufs=ctx_bufs, space="PSUM"
            ) as psum_ctx,
            tc.tile_pool(name="psum_rbz", bufs=1, space="PSUM") as psum_rbz,
        ):
            sq = getattr(nc, smalls_q)
            wq = getattr(nc, w1_q)
            blob_sb = singles.tile([P, NBLOB], F32)
            sq.dma_start(blob_sb, blob[:, :])

            def pen_col(b):
                return blob_sb[:, b * SK:(b + 1) * SK]

            def cb_col(m, b):
                c0 = 64 + m * bpc + b
                return blob_sb[:, c0:c0 + 1]

            vt_sb = singles.tile([P, MH], F16)
            nc.vector.tensor_copy(vt_sb, blob_sb[:, 96:96 + MH])

            def vt_col(m):
                return vt_sb[:, m:m + 1]

            oc_sb = blob_sb[:, 104:105]
            or_sb = blob_sb[0:1, 112:240]
            w1_sb = singles.tile([P, KD, h], F8)
            w1_r = w1.rearrange("(ko p) hh -> p ko hh", p=P)
            if not head_ilv:
                for k in range(0, KD, 4):
                    wq.dma_start(w1_sb[:, k:k + 4, :], w1_r[:, k:k + 4, :])

            et_tiles = {}    # (b, hf) -> tile
            en_tiles = {}    # (b, t) -> tile
            encn_tiles = {}  # (b, j) -> tile
            psc_of = {}      # b -> psum scores tile
            p1_done = set()

            def load_et(b, hf):
                et = et_pool.tile([P, KD, sh], F8, tag="et")
                for kc in range(0, KD, etch):
                    if head_ilv and b == 0 and hf == 0:
                        # pair each w1 k-chunk with the matching et chunk
                        # on the same queue so the head's paced m-groups
                        # unlock steadily
                        wq.dma_start(
                            w1_sb[:, kc:kc + etch, :],
                            w1_r[:, kc:kc + etch, :],
                        )
                    nc.sync.dma_start(
                        et[:, kc:kc + etch, :],
                        enc8[
                            b, kc * P:(kc + etch) * P, hf * sh:(hf + 1) * sh
                        ].rearrange("(ko p) t -> p ko t", p=P),
                    )
                et_tiles[(b, hf)] = et

            def load_encn(b, j):
                ent = encn_pool.tile([P, skg, d], F83, tag="ent")
                nc.scalar.dma_start(
                    ent,
                    encn[
                        b, j * skg * P:(j + 1) * skg * P, :
                    ].rearrange("(c p) dd -> p c dd", p=P),
                )
                encn_tiles[(b, j)] = ent

            def emit_p1(b, t):
                if (b, t) in p1_done:
                    return
                p1_done.add((b, t))
                hf, n = t // NTH, t % NTH
                if (b, hf) not in et_tiles:
                    load_et(b, hf)
                et = et_tiles[(b, hf)]
                en = en_pool.tile([P, MH, nt], F16, tag="en")
                for m in range(MH):
                    ps = psum_mm.tile([P, nt], F32, tag="mm")
                    for c in range(KD2):
                        nc.tensor.matmul(
                            ps,
                            w1_sb[:, 2 * c:2 * c + 2, m * P:(m + 1) * P],
                            et[:, 2 * c:2 * c + 2, n * nt:(n + 1) * nt],
                            start=(c == 0),
                            stop=(c == KD2 - 1),
                            perf_mode=PM.DoubleRow,
                        )
                    nc.scalar.activation(
                        en[:, m, :], ps, AF.Tanh,
                        bias=cb_col(m, b), scale=1.0 / (SE * SW),
                    )
                en_tiles[(b, t)] = en
                if t == NTH - 1 and hf == 0:
                    # queue the second half's loads right after the first
                    # half's last tile is emitted
                    pass

            def emit_sc(b, t):
                if b not in psc_of:
                    psc_of[b] = psum_sc.tile(
                        [P, SK], F32, tag="psc", name="psc"
                    )
                psc = psc_of[b]
                en = en_tiles.pop((b, t))
                for j in range(nt // P):
                    sk = t * (nt // P) + j
                    for m in range(MH):
                        nc.tensor.matmul(
                            psc[:, sk:sk + 1],
                            en[:, m, j * P:(j + 1) * P],
                            vt_col(m),
                            start=(m == 0),
                            stop=(m == MH - 1),
                        )

            attn_of = {}
            rb_of = {}

            def emit_softmax(b):
                # softmax (no max subtraction: scores are small by
                # construction; masked tokens have -1e30 penalty -> exp 0)
                sc_sb = sc_pool.tile([P, SK], F32, tag="sc")
                nc.vector.tensor_tensor(
                    sc_sb, psc_of.pop(b), pen_col(b), ALU.add
                )
                attn = attn_pool.tile([P, SK], F16, tag="attn")
                esum = stat_pool.tile([P, 1], F32, tag="esum")
                nc.scalar.activation(
                    attn, sc_sb, AF.Exp, scale=1.0, accum_out=esum
                )
                rbz = psum_rbz.tile([P, 2], F32, tag="rbz")
                # Z = sum over partitions (ones-column stationary)
                nc.tensor.matmul(rbz[0:1, 1:2], oc_sb, esum)
                rinv = stat_pool.tile([1, 1], F32, tag="rinv")
                nc.vector.reciprocal(rinv, rbz[0:1, 1:2])
                # broadcast 1/Z to all 128 partitions; park it in SBUF so
                # the psum slot recycles before the (possibly much later)
                # pass2 consumes it
                nc.tensor.matmul(rbz[:, 0:1], or_sb, rinv)
                rb = stat_pool.tile([P, 1], F32, tag="rb", bufs=4)
                nc.vector.tensor_copy(rb, rbz[:, 0:1])
                attn_of[b] = attn
                rb_of[b] = rb

            def emit_pass2(b):
                attn = attn_of.pop(b)
                ctxacc = ctxa_pool.tile([P, SK], F32, tag="ctxa")
                for j in range(NJ):
                    ent = encn_tiles.pop((b, j))
                    ctp = psum_ctx.tile([P, SK], F32, tag="ctp")
                    for cd in range(SK):
                        for q in range(skg):
                            sk = j * skg + q
                            nc.tensor.matmul(
                                ctp[:, cd:cd + 1],
                                ent[:, q, cd * P:(cd + 1) * P],
                                attn[:, sk:sk + 1],
                                start=(q == 0),
                                stop=(q == skg - 1),
                            )
                    if j == 0:
                        nc.vector.tensor_copy(ctxacc, ctp)
                    else:
                        nc.vector.tensor_tensor(ctxacc, ctxacc, ctp, ALU.add)
                ctxo = ctxo_pool.tile([P, SK], F32, tag="ctxo")
                nc.vector.tensor_scalar_mul(ctxo, ctxacc, rb_of.pop(b))
                nc.sync.dma_start(ctx_out[b], ctxo)

            # Global tile schedule: pass1 runs LAG tiles ahead of the
            # flipped scores matmuls so ScalarE's tanh latency (and the
            # per-batch softmax chain) hides under queued pass1 work.
            LAG = lag
            tiles = [(b, t) for b in range(bpc) for t in range(NT)]

            def emit_done(i):
                bb, tt = tiles[i]
                emit_sc(bb, tt)
                if tt == NT - 1:
                    emit_softmax(bb)
                    emit_pass2(bb)

            for i, (b, t) in enumerate(tiles):
                emit_p1(b, t)
                if t >= NT - 2:
                    # queue this batch's natural-layout loads after both
                    # transposed halves are on the SP queue
                    for _j in range((NJ // 2) * (t - (NT - 2)),
                                    (NJ // 2) * (t - (NT - 2) + 1)):
                        load_encn(b, _j)
                if i >= LAG:
                    emit_done(i - LAG)
            for i in range(len(tiles) - LAG, len(tiles)):
                emit_done(i)
    nc.finalize()
    return nc


_PROGRAM_CACHE = {}


def _get_program(key="full", **kwargs):
    if key not in _PROGRAM_CACHE:
        _PROGRAM_CACHE[key] = build_program(**kwargs)
    return _PROGRAM_CACHE[key]


def prep_inputs(enc_output, enc_mask, dec_hidden, W_w, W_b, V_w, V_b):
    """Host-side shard + prep: returns per-core in_maps."""
    P, SK = 128, S // 128
    enc = np.asarray(enc_output, dtype=np.float32)                 # (B,S,D)
    mask = np.asarray(enc_mask, dtype=np.float32)[..., 0]          # (B,S)
    dec = np.asarray(dec_hidden, dtype=np.float32)[0]              # (B,H)
    W = np.asarray(W_w, dtype=np.float32)                          # (H,3H)
    Wb = np.asarray(W_b, dtype=np.float32)                         # (H,)
    V = np.asarray(V_w, dtype=np.float32)[0]                       # (H,)
    Vb = float(np.asarray(V_b, dtype=np.float32)[0])

    w1t = np.ascontiguousarray(W[:, :D].T)                         # (D,H)
    cb = dec @ W[:, D:].T + Wb                                     # (B,H)

    W8 = (w1t * SW).astype(F8NP)                                   # (D,H)
    dW = W8.astype(np.float32) / SW - w1t                          # (D,H)

    # cbar[b,h] = E_s[tanh'(X_sh)] with X_sh ~ N(cb[b,h], sig^2),
    # via Gauss-Hermite quadrature.
    sig = float(np.sqrt(
        (enc.astype(np.float64) ** 2).mean()
        * (w1t.astype(np.float64) ** 2).sum(0).mean()
    ))
    xs, ws = np.polynomial.hermite.hermgauss(21)
    wsn = ws / np.sqrt(np.pi)
    z = np.sqrt(2.0) * sig * xs[None, None, :] + cb[:, :, None]
    cbar = (wsn * (1.0 / np.cosh(z) ** 2)).sum(-1).astype(np.float32)

    pen_all = np.where(mask > 0, 0.0, -1e30).astype(np.float32) + Vb
    vc = V[None, :] * cbar                                         # (B,H)
    gW = dW @ vc.T                                                 # (D,B)
    gE = w1t @ vc.T                                                # (D,B)

    enc8T = np.empty((B, D, S), dtype=F8NP)
    mx = float(ml_dtypes.finfo(F83NP).max)
    encn8 = np.clip(enc * SN, -mx, mx).astype(F83NP)               # (B,S,D)
    for b in range(B):
        e8b = (enc[b] * SE).astype(F8NP)                           # (S,D)
        enc8T[b] = e8b.T
        e_dev = e8b.astype(np.float32) * (1.0 / SE)                # (S,D)
        corr = e_dev @ (gW[:, b] + gE[:, b]) - enc[b] @ gE[:, b]
        pen_all[b] -= corr
    pen_dev = np.ascontiguousarray(
        pen_all.reshape(B, SK, P).transpose(0, 2, 1)               # (B,P,SK)
    )

    MH = H // P
    in_maps = []
    for c in range(NCORES):
        sl = slice(c * BPC, (c + 1) * BPC)
        blob = np.zeros((P, 256), dtype=np.float32)
        # pen: cols [0 : BPC*SK), batch-major
        blob[:, :BPC * SK] = pen_dev[sl].transpose(1, 0, 2).reshape(P, -1)
        # cbias: cols [64 : 64+MH*BPC), m-major; cb_sb[p, m, b] = cb[b, m*P+p]
        cb_core = cb[sl].T.reshape(MH, P, BPC).transpose(1, 0, 2)  # (P,MH,BPC)
        blob[:, 64:64 + MH * BPC] = cb_core.reshape(P, -1)
        # V chunks: cols [96 : 96+MH); vt[p, m] = V[m*P+p]
        blob[:, 96:96 + MH] = V.reshape(MH, P).T
        blob[:, 104] = 1.0          # ones column (partition reduce)
        # "ones row": stationary of the 1/Z broadcast; carries the 1/SN
        # descale of the e3m4 natural-layout enc, so pass2's final
        # tensor_scalar_mul applies 1/(Z*SN) in one shot
        blob[:, 112:240] = 1.0 / SN
        in_maps.append({
            "enc8": enc8T[sl],
            "w1": W8,
            "blob": blob,
            "encn": encn8[sl],
        })
    return in_maps


def kernel(**inputs) -> np.ndarray:
    in_maps = prep_inputs(**inputs)
    nc = _get_program("full")
    res = run_bass_kernel_spmd(nc, in_maps, list(range(NCORES)))
    out = np.concatenate(
        [res.results[c]["ctx"] for c in range(NCORES)], axis=0
    )                                           # (B, P, D//P)
    out = out.transpose(0, 2, 1).reshape(B, D)  # d = c*128 + p
    return np.ascontiguousarray(out.astype(np.float32))


if __name__ == "__main__":
    rng = np.random.default_rng(0)
    inputs = {
        "enc_output": rng.standard_normal((B, S, D), dtype=np.float32),
        "enc_mask": np.ones((B, S, 1), dtype=np.float32),
        "dec_hidden": rng.standard_normal((1, B, H), dtype=np.float32),
        "W_w": (rng.standard_normal((H, 3 * H), dtype=np.float32)
                / np.sqrt(3 * H)),
        "W_b": np.zeros((H,), dtype=np.float32),
        "V_w": rng.standard_normal((1, H), dtype=np.float32) / np.sqrt(H),
        "V_b": np.zeros((1,), dtype=np.float32),
    }
    out = kernel(**inputs)
    print(out.shape, out.dtype, float(np.abs(out).mean()))
